# revision 55
# baseline (speedup 1.0000x reference)
"""Trainium2 Bass kernel for nn_EquivariantProductBasisBlock.

Computation (per node n, channel c):
  s = nf[n,c,0]; v = nf[n,c,1:4]; v2 = |v|^2
  out0 = w0*s + w1*s^2 + w2'*v2 + w3*s^3 + w4*s*v2     (w_p = W0[sp[n],p,c])
  B1   = u0 + u1'*s + u2'*s^2 + u3'*v2                 (u_p = W1[sp[n],p,c])
  o1m  = B1 * v_m
  y0 = out0 @ L0 / sqrt(C);  y1m = o1m @ L1 / sqrt(C)
  y[n,c,:] = [y0, y1x, y1y, y1z] + sc[n,c,:]

Strategy: data-parallel over nodes across 8 cores.  On the host, nodes
are SORTED BY SPECIES and padded so every 512-node tile is
single-species.  The per-(species,path,channel) weights then become
per-partition f32 scalar columns in a small table, so the one-hot
gather matmuls, their PSUM downcast, and all transposes disappear:

  - inputs arrive as transposed bf16 planes (channels on partitions),
    pair-major [tpair, 4, C, 1024] plus an optional single-tile tail
  - Horner middle on DVE via tensor_scalar with per-partition f32
    coefficient APs (4x mode); squares on ACT; |v|^2 adds on GPSIMD;
    all chain hops stay inside DVE (no cross-engine ping-pong)
  - two-stage software pipeline: input-only ops (squares, coefficient
    FMAs, v2) run one group ahead of the dependent products
  - channel mixing computed transposed: yT[d,n] = sum_c L[c,d] X[c,n],
    i.e. matmul(lhsT=L, rhs=X) - no transposes anywhere
  - sc (also transposed bf16 planes) is injected into the same PSUM
    accumulation via identity matmuls, issued FIRST so the PE p-state
    stays ramped and PSUM recycles early
  - PSUM -> SBUF bf16 copies on ACT (GPSIMD cannot access PSUM),
    drained one group behind the compute; bf16 DMAs throughout
  - host reassembles: inverse node permutation + plane interleave
"""

import numpy as np

N_CORES = 8
N_NODES = 65536
C = 128
E = 10
W = 512          # nodes per sub-tile (one PSUM bank per output plane)

INV_SQ3 = 1.0 / np.sqrt(3.0)
SQ2 = float(np.sqrt(2.0))
SQ3 = float(np.sqrt(3.0))
SQ35 = float(np.sqrt(3.0 / 5.0))

_CACHE = {}


# ---------------------------------------------------------------------------
# Workarounds for the walrus build in this container: it rejects any
# instruction carrying more than one sync-wait ("Too many sync wait
# commands").  Split extra waits onto same-engine NOPs preceding the
# instruction (identical semantics: the engine queue is FIFO).
# ---------------------------------------------------------------------------
def _apply_patches():
    import concourse.tile as tile
    from concourse import mybir
    from concourse.vector_clock import ScopedClock

    if getattr(tile.TileContext, "_singlewait_patched", False):
        return

    def _patched_drain_and_barrier(self, tick_clock, wait_clock):
        nc = self.nc
        probe = nc.sync.nop()
        wait_clock.add_sem_waits(probe.ins, ScopedClock({None: tick_clock.global_clock}))
        si = probe.ins.sync_info
        waits = list(si.on_wait) if si and si.on_wait else []
        if len(waits) > 1:
            probe.ins.sync_info = type(si)(on_wait=waits[:1], on_update=[])
            for w in waits[1:]:
                extra = nc.sync.nop()
                extra.ins.sync_info = type(si)(on_wait=[w], on_update=[])
        nc.sync.drain()
        nc.all_engine_barrier()
        assert self.sems is not None
        popped = nc._tile_sem_poison_stack.pop()
        assert popped is self._sem_poison
        nc.clear_and_free_semaphores(list(self.sems.allocated().values()))
        nc.all_engine_barrier()

    _orig_commit = tile.TileContext._commit_instruction

    def _split_commit(self, inst, lazy_reg_writes=True):
        si = getattr(inst, "sync_info", None)
        if (si is not None and si.on_wait and len(si.on_wait) > 1
                and getattr(inst, "engine", mybir.EngineType.Unassigned)
                != mybir.EngineType.Unassigned):
            waits = list(si.on_wait)
            for w in waits[:-1]:
                nop = mybir.InstNoOp(name=self.nc.get_next_instruction_name(),
                                     ins=[], outs=[], engine=inst.engine)
                nop.sync_info = mybir.SyncInfo(on_wait=[w], on_update=[])
                _orig_commit(self, nop, lazy_reg_writes=False)
            inst.sync_info = mybir.SyncInfo(on_wait=[waits[-1]],
                                            on_update=list(si.on_update or []))
        return _orig_commit(self, inst, lazy_reg_writes)

    tile.TileContext._drain_and_barrier = _patched_drain_and_barrier
    tile.TileContext._commit_instruction = _split_commit
    tile.TileContext._singlewait_patched = True


def _build_program(reps=1, t_core=17):
    import concourse.bass as bass
    import concourse.tile as tile
    from concourse import mybir
    from concourse.masks import make_identity
    from contextlib import ExitStack

    _apply_patches()
    F32 = mybir.dt.float32
    BF16 = mybir.dt.bfloat16
    F8 = mybir.dt.float8e4
    AF = mybir.ActivationFunctionType
    nc = bass.Bass()

    t_half = t_core // 2
    has_tail = t_core % 2 == 1
    W2 = 2 * W
    n_elems = t_core * 4 * C * W
    xin_d = nc.declare_dram_parameter("xin", [n_elems], BF16, isOutput=False)
    sct_d = nc.declare_dram_parameter("sct", [n_elems], F8, isOutput=False)
    wtab_d = nc.declare_dram_parameter("wtab", [C, t_core * 9], F32, isOutput=False)
    l0_d = nc.declare_dram_parameter("l0", [C, C], BF16, isOutput=False)
    l1_d = nc.declare_dram_parameter("l1", [C, C], BF16, isOutput=False)
    id8_d = nc.declare_dram_parameter("id8", [C, C], F8, isOutput=False)
    y_d = nc.declare_dram_parameter("yout", [n_elems], BF16, isOutput=True)

    mult = mybir.AluOpType.mult
    add = mybir.AluOpType.add

    def dview(dparam, gi, G):
        # DMA-side view of group gi, iteration order (c, plane, w)
        a = dparam[:]
        if G == 2:
            off = gi * (4 * C * W2)
            dims = [[W2, C], [C * W2, 4], [1, W2]]
        else:
            off = t_half * (4 * C * W2)
            dims = [[W, C], [C * W, 4], [1, W]]
        import concourse.bass as bass
        return bass.AP(tensor=a.tensor, offset=a.offset + off, ap=dims)

    with tile.TileContext(nc) as tc, ExitStack() as ctx:
        consts = ctx.enter_context(tc.tile_pool(name="consts", bufs=1))
        io = ctx.enter_context(tc.tile_pool(name="io", bufs=4))
        work = ctx.enter_context(tc.tile_pool(name="work", bufs=2))
        psY = ctx.enter_context(tc.tile_pool(name="psY", bufs=1, space="PSUM"))

        def ap(t, off, *dims):
            return bass.AP(tensor=t.tensor, offset=t.offset + off,
                           ap=[t.ap[0], *list(dims)])

        # groups: pairs of 512-node tiles (+ single tail if t_core is odd).
        # species-independent ops run G*512 wide; coefficient TS ops per
        # sub-tile.  For G == 1 the pair-shaped tiles are used with strided
        # two/three-chunk APs (innermost stays packed -> DVE fast modes).
        base = [[2 * i, 2 * i + 1] for i in range(t_half)]
        if has_tail:
            if reps > 1:
                # repeated (timing) program: the single-tile group FIRST -
                # its cadence break lands in each rep's fill phase instead
                # of the steady loop
                base.insert(0, [t_core - 1])
            else:
                # single pass: tail LAST so the final drain chain is the
                # small group
                base.append([t_core - 1])
        groups = [g for _ in range(reps) for g in base]
        ins = {}
        pend = {}
        st = {}
        Wg = W2

        def lanes(t, off, n, G):
            # n logical lanes starting at tile-offset `off`
            if G == 2:
                return ap(t, off, [1, n * Wg]) if n > 1 else t[:, off:off + Wg]
            return ap(t, off, [Wg, n], [1, W]) if n > 1 else t[:, off:off + W]

        def load(i):
            g = groups[i]
            G = len(g)
            a = io.tile([C, 4 * Wg], BF16, tag="in")
            b = io.tile([C, 4 * Wg], F8, tag="sc")
            if G == 2:
                nc.sync.dma_start(out=a, in_=dview(xin_d, g[0] // 2, 2))
                nc.sync.dma_start(out=b, in_=dview(sct_d, g[0] // 2, 2))
            else:
                nc.sync.dma_start(out=ap(a, 0, [Wg, 4], [1, W]),
                                  in_=dview(xin_d, 0, 1))
                nc.sync.dma_start(out=ap(b, 0, [Wg, 4], [1, W]),
                                  in_=dview(sct_d, 0, 1))
            ins[i] = (a, b)

        def drain(j):
            # PSUM -> SBUF copies on ACT only (GPSIMD cannot access PSUM,
            # DVE is the busiest engine)
            p_ys, g, t_sc = pend.pop(j)
            G = len(g)
            t_y = io.tile([C, 4 * Wg], BF16, tag="y")
            for k in range(G):
                p_y = p_ys[k]
                nc.scalar.activation(out=ap(t_y, k * W, [1, W]),
                                     in_=p_y[:, 0:W], func=AF.Copy)
                nc.scalar.activation(out=ap(t_y, Wg + k * W, [Wg, 3], [1, W]),
                                     in_=p_y[:, W:4 * W], func=AF.Copy)
            if G == 2:
                nc.sync.dma_start(out=dview(y_d, g[0] // 2, 2), in_=t_y)
            else:
                nc.sync.dma_start(out=dview(y_d, 0, 1),
                                  in_=ap(t_y, 0, [Wg, 4], [1, W]))

        # Horner (all chain hops stay inside DVE):
        #   out0 = ((w3*s + w1)*s + w0)*s + (w4*s + w2')*v2 = D + R
        #   B1   = (u2'*s + u1')*s + (u3'*v2 + u0)          = G + H
        # stage1(j): ops needing only in(j) [SQ3 on ACT; TS on DVE; v2
        # adds on GPSIMD gated on SQ3]. stage2(i): cross-engine deps are
        # one period old.
        def stage1(j):
            t_in, _ = ins[j]
            g = groups[j]
            G = len(g)
            col = lambda k, c: t_wtab[:, g[k] * 9 + c:g[k] * 9 + c + 1]
            d = {}
            t_sq = work.tile([C, 3 * Wg], BF16, tag="sq")   # [vx2|vy2|vz2]
            t_v2 = work.tile([C, Wg], BF16, tag="v2")
            t_s1 = work.tile([C, 3 * Wg], BF16, tag="s1")   # [A | F | Q]
            d.update(sq=t_sq, v2=t_v2, s1=t_s1)
            nc.scalar.activation(out=lanes(t_sq, 0, 3, G),
                                 in_=lanes(t_in, Wg, 3, G), func=AF.Square)
            for k in range(G):
                Sk = t_in[:, k * W:(k + 1) * W]
                nc.vector.tensor_scalar(out=ap(t_s1, k * W, [1, W]), in0=Sk,
                                        scalar1=col(k, 0), scalar2=col(k, 1),
                                        op0=mult, op1=add)
                nc.vector.tensor_scalar(out=ap(t_s1, Wg + k * W, [1, W]),
                                        in0=Sk, scalar1=col(k, 5),
                                        scalar2=col(k, 6), op0=mult, op1=add)
                nc.vector.tensor_scalar(out=ap(t_s1, 2 * Wg + k * W, [1, W]),
                                        in0=Sk, scalar1=col(k, 3),
                                        scalar2=col(k, 4), op0=mult, op1=add)
            nc.gpsimd.tensor_tensor(out=lanes(t_v2, 0, 1, G),
                                    in0=lanes(t_sq, 0, 1, G),
                                    in1=lanes(t_sq, Wg, 1, G), op=add)
            nc.gpsimd.tensor_tensor(out=lanes(t_v2, 0, 1, G),
                                    in0=lanes(t_v2, 0, 1, G),
                                    in1=lanes(t_sq, 2 * Wg, 1, G), op=add)
            st[j] = d

        load(0)
        t_wtab = consts.tile([C, t_core * 9], F32)
        nc.sync.dma_start(out=t_wtab, in_=wtab_d[:, :])
        t_l0 = consts.tile([C, C], BF16)
        nc.sync.dma_start(out=t_l0, in_=l0_d[:, :])
        t_l1 = consts.tile([C, C], BF16)
        nc.sync.dma_start(out=t_l1, in_=l1_d[:, :])
        ident = consts.tile([C, C], F8)
        nc.sync.dma_start(out=ident, in_=id8_d[:, :])
        for j in range(1, min(3, len(groups))):
            load(j)
        for i, g in enumerate(groups):
            G = len(g)
            col = lambda k, c: t_wtab[:, g[k] * 9 + c:g[k] * 9 + c + 1]
            # cols: 0:w3 1:w1 2:w0 3:w4 4:w2' 5:u2' 6:u1' 7:u3' 8:u0

            if i + 3 < len(groups):
                load(i + 3)
            if i == 0:
                stage1(0)
            t_in, t_sc = ins.pop(i)
            d = st.pop(i)
            t_v2 = d["v2"]
            t_s1 = d["s1"]

            # --- [B|G] = [A|F] * s ; C = B + w0 (over B) ; D = C*s ---
            t_bg = work.tile([C, 2 * Wg], BF16, tag="bg")
            srep = (ap(t_in, 0, [0, 2], [1, Wg]) if G == 2
                    else ap(t_in, 0, [0, 2], [Wg, 1], [1, W]))
            nc.vector.tensor_tensor(out=lanes(t_bg, 0, 2, G),
                                    in0=lanes(t_s1, 0, 2, G), in1=srep,
                                    op=mult)
            for k in range(G):
                nc.vector.tensor_scalar(out=ap(t_bg, k * W, [1, W]),
                                        in0=t_bg[:, k * W:(k + 1) * W],
                                        scalar1=col(k, 2), scalar2=None,
                                        op0=add)
            nc.vector.tensor_tensor(out=lanes(t_bg, 0, 1, G),
                                    in0=lanes(t_bg, 0, 1, G),
                                    in1=lanes(t_in, 0, 1, G), op=mult)

            # drain i-1 now: ACT copies run before SQ3(i+1) so PSUM banks
            # recycle early and this group's sc matmuls can start
            if i - 1 in pend:
                drain(i - 1)
            # prefetch next group's independent stage
            if i + 1 in ins:
                stage1(i + 1)

            # --- R = Q*v2 ; H = u3'*v2+u0 ; [out0|B1] = [D|G]+[R|H] ---
            t_rh = work.tile([C, 2 * Wg], BF16, tag="rh")
            nc.vector.tensor_tensor(out=lanes(t_rh, 0, 1, G),
                                    in0=lanes(t_s1, 2 * Wg, 1, G),
                                    in1=lanes(t_v2, 0, 1, G), op=mult)
            for k in range(G):
                nc.vector.tensor_scalar(out=ap(t_rh, Wg + k * W, [1, W]),
                                        in0=t_v2[:, k * W:(k + 1) * W],
                                        scalar1=col(k, 7), scalar2=col(k, 8),
                                        op0=mult, op1=add)
            t_ob = work.tile([C, 2 * Wg], BF16, tag="ob")
            nc.vector.tensor_tensor(out=lanes(t_ob, 0, 2, G),
                                    in0=lanes(t_bg, 0, 2, G),
                                    in1=lanes(t_rh, 0, 2, G), op=add)
            # --- O1 = B1 * v ---
            t_o1 = work.tile([C, 3 * Wg], BF16, tag="o1")
            b1rep = (ap(t_ob, Wg, [0, 3], [1, Wg]) if G == 2
                     else ap(t_ob, Wg, [0, 3], [Wg, 1], [1, W]))
            nc.vector.tensor_tensor(out=lanes(t_o1, 0, 3, G),
                                    in0=lanes(t_in, Wg, 3, G), in1=b1rep,
                                    op=mult)

            # --- channel mixing, transposed: yT = L^T X (+ I^T scT) ---
            # sc identity matmuls first: they only need t_sc + freed PSUM,
            # so they start early and keep the PE p-state ramped before the
            # L matmuls; grouped by lhsT (3 weight loads per group)
            p_y0 = psY.tile([C, 4 * W], F32, tag="py0")
            p_ys = [p_y0]
            if G == 2:
                p_y1 = psY.tile([C, 4 * W], F32, tag="py1")
                p_ys.append(p_y1)
            for k in range(G):
                for m in range(4):
                    nc.tensor.matmul(p_ys[k][:, m * W:(m + 1) * W], lhsT=ident,
                                     rhs=t_sc[:, m * Wg + k * W:m * Wg + (k + 1) * W],
                                     start=True, stop=False)
            for k in range(G):
                nc.tensor.matmul(p_ys[k][:, 0:W], lhsT=t_l0,
                                 rhs=t_ob[:, k * W:(k + 1) * W],
                                 start=False, stop=True)
            for k in range(G):
                for m in range(3):
                    nc.tensor.matmul(p_ys[k][:, (1 + m) * W:(2 + m) * W],
                                     lhsT=t_l1,
                                     rhs=t_o1[:, m * Wg + k * W:m * Wg + (k + 1) * W],
                                     start=False, stop=True)
            pend[i] = (p_ys, g, t_sc)
            if i == len(groups) - 1:
                drain(i)
        assert not pend

    return nc


def _prep_host(inputs):
    import ml_dtypes
    bf16 = ml_dtypes.bfloat16

    nf = np.asarray(inputs["node_feats"], dtype=np.float32)
    sc = np.asarray(inputs["sc"], dtype=np.float32)
    sp = np.asarray(inputs["node_species"]).astype(np.int64)
    W0 = np.asarray(inputs["W0"], dtype=np.float32)
    W1 = np.asarray(inputs["W1"], dtype=np.float32)
    L0 = np.asarray(inputs["L0"], dtype=np.float32)
    L1 = np.asarray(inputs["L1"], dtype=np.float32)

    n = nf.shape[0]
    perm = np.argsort(sp, kind="stable")
    sp_sorted = sp[perm]
    counts = np.bincount(sp, minlength=E)
    tiles_e = (counts + W - 1) // W
    t_total = int(tiles_e.sum())
    t_core = max(1, -(-t_total // N_CORES))
    t_pad = N_CORES * t_core
    npad = t_pad * W

    slot_off = np.zeros(E + 1, dtype=np.int64)
    slot_off[1:] = np.cumsum(tiles_e) * W
    cum_counts = np.zeros(E + 1, dtype=np.int64)
    cum_counts[1:] = np.cumsum(counts)
    idx_within = np.arange(n, dtype=np.int64) - cum_counts[sp_sorted]
    slots = slot_off[sp_sorted] + idx_within  # padded slot of sorted node k

    nf_pad = np.zeros((npad, C, 4), dtype=bf16)
    nf_pad[slots] = nf[perm].astype(bf16)
    sc_pad = np.zeros((npad, C, 4), dtype=bf16)
    sc_pad[slots] = sc[perm].astype(bf16)

    # per-core flat layout: pair-major planes + optional single-tile tail
    t_half = t_core // 2

    def to_flat(arr):
        cores = []
        for cidx in range(N_CORES):
            blk = arr[cidx * t_core * W:(cidx + 1) * t_core * W]
            pairs = blk[:t_half * 2 * W].reshape(t_half, 2 * W, C, 4)
            parts = [np.ascontiguousarray(pairs.transpose(0, 3, 2, 1)).ravel()]
            if t_core % 2 == 1:
                tail = blk[t_half * 2 * W:]          # [W, C, 4]
                parts.append(
                    np.ascontiguousarray(tail.transpose(2, 1, 0)).ravel())
            cores.append(np.concatenate(parts))
        return cores

    import ml_dtypes as _md
    xin = to_flat(nf_pad)
    sct = [a.astype(_md.float8_e4m3) for a in to_flat(sc_pad)]

    # per-tile species (padding tiles -> coefficient zeros)
    tile_species = np.full(t_pad, -1, dtype=np.int64)
    ti = 0
    for e in range(E):
        tile_species[ti:ti + tiles_e[e]] = e
        ti += int(tiles_e[e])

    # coefficient columns per tile: [w3, w1, w0, w4, w2', u2', u1', u3', u0]
    coef = np.zeros((E + 1, 9, C), dtype=np.float32)  # row E stays zero (pad)
    coef[:E, 0] = W0[:, 3]
    coef[:E, 1] = W0[:, 1]
    coef[:E, 2] = W0[:, 0]
    coef[:E, 3] = W0[:, 4]
    coef[:E, 4] = W0[:, 2] * INV_SQ3
    coef[:E, 5] = W1[:, 2] * SQ3
    coef[:E, 6] = W1[:, 1] * SQ2
    coef[:E, 7] = W1[:, 3] * SQ35
    coef[:E, 8] = W1[:, 0]
    tile_coef = coef[tile_species]                    # [t_pad, 9, C]
    wtab = np.ascontiguousarray(
        tile_coef.reshape(t_pad, 9, C).transpose(2, 0, 1).reshape(C, t_pad * 9))

    inv_sqrt_c = np.float32(1.0 / np.sqrt(C))
    l0 = np.ascontiguousarray((L0 * inv_sqrt_c).astype(bf16))
    l1 = np.ascontiguousarray((L1 * inv_sqrt_c).astype(bf16))
    id8 = np.eye(C, dtype=_md.float8_e4m3)

    meta = dict(perm=perm, slots=slots, t_core=t_core, t_pad=t_pad, n=n,
                id8=id8)
    return xin, sct, wtab, l0, l1, meta


def _in_maps(xin, sct, wtab, l0, l1, meta):
    t_core = meta["t_core"]
    maps = []
    for cidx in range(N_CORES):
        lo, hi = cidx * t_core, (cidx + 1) * t_core
        maps.append({
            "xin": xin[cidx],
            "sct": sct[cidx],
            "wtab": np.ascontiguousarray(wtab[:, lo * 9:hi * 9]),
            "l0": l0,
            "l1": l1,
            "id8": meta["id8"],
        })
    return maps


def _assemble(y_cores, meta):
    t_pad, n, t_core = meta["t_pad"], meta["n"], meta["t_core"]
    t_half = t_core // 2
    parts = []
    for y in y_cores:                                # flat [t_core*4*C*W]
        pairs = y[:t_half * 4 * C * 2 * W].reshape(t_half, 4, C, 2 * W)
        parts.append(pairs.transpose(0, 3, 2, 1).reshape(-1, C, 4))
        if t_core % 2 == 1:
            tail = y[t_half * 4 * C * 2 * W:].reshape(4, C, W)
            parts.append(tail.transpose(2, 1, 0))
    y = np.concatenate(parts, axis=0).astype(np.float32)  # [t_pad*W, C, 4]
    out = np.empty((n, C, 4), dtype=np.float32)
    out[meta["perm"]] = y[meta["slots"]]
    return out


def kernel(**inputs):
    from concourse.bass_utils import run_bass_kernel_spmd

    xin, sct, wtab, l0, l1, meta = _prep_host(inputs)
    t_core = meta["t_core"]
    key = ("nc", t_core)
    if key not in _CACHE:
        _CACHE[key] = _build_program(t_core=t_core)
    nc = _CACHE[key]

    res = run_bass_kernel_spmd(nc, _in_maps(xin, sct, wtab, l0, l1, meta),
                               core_ids=list(range(N_CORES)))
    _CACHE["last_result"] = res
    y_cores = [res.results[c]["yout"] for c in range(N_CORES)]
    return _assemble(y_cores, meta)
